# revision 5
# baseline (speedup 1.0000x reference)
"""GIN message-passing (CentralityChannel) on 8 trn2 NeuronCores.

Strategy (graph/data parallel per sharding hint):
  - Nodes sharded: core c owns rows [c*6250, (c+1)*6250), padded to 6272 = 49
    tiles of 128. The full node-feature table h [50176, 64] is replicated in
    every core's DRAM and rebuilt each layer with an AllGather.
  - Edges sharded by TARGET core. Per target tile (128 nodes), edges are
    grouped into blocks of 128, split by source-table half (dma_gather uses
    int16 indices < 32768), zero-weight padded to static block counts
    (BLO lo-blocks + BHI hi-blocks per tile, same for every core).
  - Per block: gathered rows [128e, 64f] are weighted (DVE), a one-hot
    S[e, n] = (tgtloc[e] == n) is built on DVE, and PE accumulates
    aggT[64f, 128n] += msg.T @ S in PSUM == segment-sum of w*h[src].
  - GIN combine is folded: edge weights are pre-divided by (1+eps_l) on the
    host and W1_l is pre-multiplied by (1+eps_l), so y0 = hT_own + aggT.
  - MLP runs feature-major: y1T = W1'.T @ y0. BatchNorm1d (training mode)
    needs global batch stats: per-channel sum/sumsq accumulate via ACT
    accum_out, AllReduce [64, 2] across cores, then BN+bias+ReLU is one ACT
    op with per-partition scale/bias. MLP biases cancel inside BatchNorm and
    are dropped.
  - New h shard is transposed back node-major via PE and AllGathered into the
    next layer's table. The final layer skips the AllGather; the host
    assembles shards and applies mask_teams.
"""

import sys

sys.path.insert(0, "/opt/trn_rl_repo")
import numpy as np

NODES, D, NL = 50000, 64, 3
NCORES = 8
NSH = NODES // NCORES            # 6250
P = 128
NT = (NSH + P - 1) // P          # 49
NLOC = NT * P                    # 6272
TB = NCORES * NLOC               # 50176
HALF = TB // 2                   # 25088
BN_EPS = 1e-5
G = 4                            # target tiles per dma_gather call
CHUNK = 512                      # node cols per MLP2 matmul

_CALLS = [list(range(s, min(s + G, NT))) for s in range(0, NT, G)]


def _plan(edge_index, edge_weight, eps):
    """Host preprocessing: shard/sort/pad edges into the static block layout."""
    src = edge_index[0].astype(np.int64)
    tgt = edge_index[1].astype(np.int64)
    w = edge_weight.astype(np.float32)
    assert np.all(np.abs(1.0 + eps) > 1e-6), "eps == -1 unsupported"

    src_row = (src // NSH) * NLOC + (src % NSH)     # row in padded table
    c_tgt = tgt // NSH
    r = tgt % NSH
    tile = r // P
    lane = (r % P).astype(np.float32)               # one-hot lane value
    half = (src_row >= HALF).astype(np.int64)

    key = (c_tgt * NT + tile) * 2 + half
    order = np.argsort(key, kind="stable")
    counts = np.bincount(key, minlength=NCORES * NT * 2)
    starts = np.zeros_like(counts)
    starts[1:] = np.cumsum(counts)[:-1]
    q = np.arange(len(src)) - starts[key[order]]    # rank within group

    BLO = int(np.ceil(counts.reshape(-1, 2)[:, 0].max() / P))
    BHI = int(np.ceil(counts.reshape(-1, 2)[:, 1].max() / P))
    BPT = BLO + BHI

    so, ho, co, to = src_row[order], half[order], c_tgt[order], tile[order]
    lo_, wo = lane[order], w[order]
    b = q // P
    p = q % P
    bcount = np.where(ho == 0, BLO, BHI)
    assert np.all(b < bcount), "block overflow; BLO/BHI too small"

    cores = []
    for c in range(NCORES):
        m = co == c
        tokv = {0: np.zeros(NT * BLO * P, np.int64),
                1: np.zeros(NT * BHI * P, np.int64)}
        wtok = {0: np.zeros(NT * BLO * P, np.float32),
                1: np.zeros(NT * BHI * P, np.float32)}
        ttok = np.zeros(NT * BPT * P, np.float32)
        for h, bp in ((0, BLO), (1, BHI)):
            mh = m & (ho == h)
            pos = (to[mh] * bp + b[mh]) * P + p[mh]
            tokv[h][pos] = so[mh] - h * HALF
            wtok[h][pos] = wo[mh]
            bb = b[mh] + (BLO if h else 0)
            ttok[(to[mh] * BPT + bb) * P + p[mh]] = lo_[mh]

        def wrap(tv, bp):
            outs = []
            for tiles in _CALLS:
                t0, t1 = tiles[0], tiles[-1] + 1
                seg = tv[t0 * bp * P:t1 * bp * P]
                outs.append(seg.reshape(-1, 16).T)
            wr = np.concatenate(outs, axis=1).astype(np.int16)
            return np.tile(wr, (8, 1))              # replicate to 128 rows

        wlo = np.stack([wtok[0] / (1.0 + eps[l]) for l in range(NL)])
        whi = np.stack([wtok[1] / (1.0 + eps[l]) for l in range(NL)])
        cores.append(dict(
            idxlo=wrap(tokv[0], BLO), idxhi=wrap(tokv[1], BHI),
            wlo=np.concatenate([a.reshape(NT * BLO, P).T for a in wlo], 1),
            whi=np.concatenate([a.reshape(NT * BHI, P).T for a in whi], 1),
            tgtloc=ttok.reshape(NT * BPT, P).T.copy(),
            tokv=tokv, wtok=wtok, ttokf=ttok,
        ))
    return dict(BLO=BLO, BHI=BHI, BPT=BPT, cores=cores)


def _tableize(x):
    rows = (np.arange(NODES) // NSH) * NLOC + np.arange(NODES) % NSH
    tb = np.zeros((TB, D), np.float32)
    tb[rows] = x
    return tb, rows


def _weights(eps, W1, W2, g1, beta1, g2, beta2):
    ws = []
    for l in range(NL):
        ws.append(((1.0 + eps[l]) * W1[l]).astype(np.float32))
        ws.append(W2[l].astype(np.float32))
    Ws = np.concatenate(ws, 0)                       # [NL*2*64, 64]
    gb = np.stack(sum([[g1[l], beta1[l], g2[l], beta2[l]] for l in range(NL)],
                      []), 1).astype(np.float32)     # [64, 12]
    return Ws, gb


def mirror(x, edge_index, edge_weight, mask_teams, eps, W1, b1, g1, beta1,
           W2, b2, g2, beta2):
    """Numpy mirror of the exact device computation (for validation)."""
    plan = _plan(np.asarray(edge_index), np.asarray(edge_weight),
                 np.asarray(eps))
    BLO, BHI, BPT = plan["BLO"], plan["BHI"], plan["BPT"]
    table, rows = _tableize(np.asarray(x))
    Ws, gb = _weights(eps, W1, W2, g1, beta1, g2, beta2)
    H = [table[c * NLOC:(c + 1) * NLOC].T.copy() for c in range(NCORES)]

    for l in range(NL):
        aggs = []
        for c in range(NCORES):
            pc = plan["cores"][c]
            msil = table[:HALF][pc["tokv"][0]] * \
                (pc["wtok"][0] / (1 + eps[l]))[:, None]
            msih = table[HALF:][pc["tokv"][1]] * \
                (pc["wtok"][1] / (1 + eps[l]))[:, None]
            mlo = msil.reshape(NT, BLO, P, D)
            mhi = msih.reshape(NT, BHI, P, D)
            msg = np.concatenate([mlo, mhi], 1)       # [NT, BPT, P, D]
            tl = pc["ttokf"].reshape(NT, BPT, P)
            S = (tl[..., None] == np.arange(P, dtype=np.float32)).astype(
                np.float32)                           # [NT, BPT, P, Pn]
            agg = np.einsum("tbpd,tbpn->dtn", msg, S).reshape(D, NLOC)
            aggs.append(agg)
        # MLP, feature-major, with cross-core BN
        y1s = []
        for c in range(NCORES):
            y0 = H[c] + aggs[c]
            y1s.append(Ws[2 * l * D:(2 * l + 1) * D].T @ y0)
        s1 = sum(y[:, :NSH].sum(1) for y in y1s)
        s1q = sum((y[:, :NSH] ** 2).sum(1) for y in y1s)
        mu, ex2 = s1 / NODES, s1q / NODES
        sc1 = gb[:, 4 * l + 0] / np.sqrt(ex2 - mu ** 2 + BN_EPS)
        bi1 = gb[:, 4 * l + 1] - mu * sc1
        y2s = []
        for c in range(NCORES):
            y1n = np.zeros_like(y1s[c])
            y1n[:, :NSH] = np.maximum(
                y1s[c][:, :NSH] * sc1[:, None] + bi1[:, None], 0)
            y2s.append(Ws[(2 * l + 1) * D:(2 * l + 2) * D].T @ y1n)
        s2 = sum(y[:, :NSH].sum(1) for y in y2s)
        s2q = sum((y[:, :NSH] ** 2).sum(1) for y in y2s)
        mu2, ex22 = s2 / NODES, s2q / NODES
        sc2 = gb[:, 4 * l + 2] / np.sqrt(ex22 - mu2 ** 2 + BN_EPS)
        bi2 = gb[:, 4 * l + 3] - mu2 * sc2
        for c in range(NCORES):
            hn = np.zeros_like(y2s[c])
            hn[:, :NSH] = np.maximum(
                y2s[c][:, :NSH] * sc2[:, None] + bi2[:, None], 0)
            H[c] = hn
            table[c * NLOC:(c + 1) * NLOC] = hn.T
    full = np.concatenate([H[c].T[:NSH] for c in range(NCORES)], 0)
    return full[np.asarray(mask_teams)]


# ---------------------------------------------------------------------------
# Device program
# ---------------------------------------------------------------------------
_cache = {}


def _build(BLO, BHI):
    from concourse import bass, bacc, mybir, tile
    from concourse.masks import make_identity

    F = mybir.dt.float32
    I16 = mybir.dt.int16
    BPT = BLO + BHI
    AL = mybir.AluOpType
    AF = mybir.ActivationFunctionType

    nc = bacc.Bacc(num_devices=NCORES)
    x_table = nc.declare_dram_parameter("x_table", [TB, D], F, isOutput=False)
    xT_own = nc.declare_dram_parameter("xT_own", [D, NLOC], F, isOutput=False)
    idxlo = nc.declare_dram_parameter("idxlo", [P, NT * BLO * 8], I16, False)
    idxhi = nc.declare_dram_parameter("idxhi", [P, NT * BHI * 8], I16, False)
    wlo_in = nc.declare_dram_parameter("wlo", [P, NL * NT * BLO], F, False)
    whi_in = nc.declare_dram_parameter("whi", [P, NL * NT * BHI], F, False)
    tloc_in = nc.declare_dram_parameter("tloc", [P, NT * BPT], F, False)
    iota_in = nc.declare_dram_parameter("iota", [P, P], F, False)
    Ws_in = nc.declare_dram_parameter("Ws", [NL * 2 * D, D], F, False)
    gb_in = nc.declare_dram_parameter("gb", [D, 4 * NL], F, False)
    h_out = nc.declare_dram_parameter("h_out", [NLOC, D], F, isOutput=True)

    cc_in = [nc.dram_tensor(f"cc_in{l}", [NLOC, D], F) for l in range(NL - 1)]
    cc_out = [nc.dram_tensor(f"cc_out{l}", [TB, D], F, addr_space="Shared")
              for l in range(NL - 1)]
    st_in = [nc.dram_tensor(f"st_in{i}", [D, 2], F) for i in range(2 * NL)]
    st_out = [nc.dram_tensor(f"st_out{i}", [D, 2], F, addr_space="Shared")
              for i in range(2 * NL)]
    rg = [list(range(NCORES))]

    with tile.TileContext(nc) as tc:
        with (
            tc.tile_pool(name="persist", bufs=1) as pp,
            tc.tile_pool(name="gat", bufs=2) as gp,
            tc.tile_pool(name="sb", bufs=3) as sb,
            tc.tile_pool(name="small", bufs=4) as sp,
            tc.tile_pool(name="ps_agg", bufs=2, space="PSUM") as ps_agg,
            tc.tile_pool(name="ps_m", bufs=2, space="PSUM") as ps_m,
            tc.tile_pool(name="ps_m2", bufs=2, space="PSUM") as ps_m2,
            tc.tile_pool(name="ps_tr", bufs=2, space="PSUM") as ps_tr,
        ):
            # resident tiles
            H = pp.tile([D, NLOC], F)
            B1 = pp.tile([D, NLOC], F)
            B2 = pp.tile([D, NLOC], F)
            ilo = pp.tile([P, NT * BLO * 8], I16)
            ihi = pp.tile([P, NT * BHI * 8], I16)
            wlo = pp.tile([P, NL * NT * BLO], F)
            whi = pp.tile([P, NL * NT * BHI], F)
            tloc = pp.tile([P, NT * BPT], F)
            iota = pp.tile([P, P], F)
            gb = pp.tile([D, 4 * NL], F)
            ident = pp.tile([D, D], F)

            nc.sync.dma_start(out=H[:], in_=xT_own[:])
            nc.sync.dma_start(out=ilo[:], in_=idxlo[:])
            nc.sync.dma_start(out=ihi[:], in_=idxhi[:])
            nc.sync.dma_start(out=wlo[:], in_=wlo_in[:])
            nc.sync.dma_start(out=whi[:], in_=whi_in[:])
            nc.sync.dma_start(out=tloc[:], in_=tloc_in[:])
            nc.sync.dma_start(out=iota[:], in_=iota_in[:])
            nc.sync.dma_start(out=gb[:], in_=gb_in[:])
            make_identity(nc, ident[:])
            nc.vector.memset(B1[:], 0.0)
            nc.vector.memset(B2[:], 0.0)
            epsc = pp.tile([D, 1], F)
            nc.vector.memset(epsc[:], BN_EPS)

            NW = NSH - (NT - 1) * P                  # 106 real cols, last tile

            for l in range(NL):
                tab = x_table if l == 0 else cc_out[l - 1]
                W1t = sp.tile([D, D], F, tag="w1")
                W2t = sp.tile([D, D], F, tag="w2")
                nc.sync.dma_start(out=W1t[:], in_=Ws_in[2 * l * D:(2 * l + 1) * D, :])
                nc.sync.dma_start(out=W2t[:], in_=Ws_in[(2 * l + 1) * D:(2 * l + 2) * D, :])
                s1 = sp.tile([D, NT], F, tag="s1")
                s1q = sp.tile([D, NT], F, tag="s1q")

                for ci, tiles in enumerate(_CALLS):
                    ntl = len(tiles)
                    t0 = tiles[0]
                    glo = gp.tile([P, G * BLO, D], F, tag="glo")
                    ghi = gp.tile([P, G * BHI, D], F, tag="ghi")
                    nc.gpsimd.dma_gather(
                        out_ap=glo[:, :ntl * BLO, :], in_ap=tab[0:HALF, :],
                        idxs_ap=ilo[:, t0 * BLO * 8:(t0 + ntl) * BLO * 8],
                        num_idxs=ntl * BLO * P, num_idxs_reg=ntl * BLO * P,
                        elem_size=D, single_packet=False)
                    nc.gpsimd.dma_gather(
                        out_ap=ghi[:, :ntl * BHI, :], in_ap=tab[HALF:TB, :],
                        idxs_ap=ihi[:, t0 * BHI * 8:(t0 + ntl) * BHI * 8],
                        num_idxs=ntl * BHI * P, num_idxs_reg=ntl * BHI * P,
                        elem_size=D, single_packet=False)
                    # weight the messages (in place)
                    nc.vector.tensor_tensor(
                        out=glo[:, :ntl * BLO, :], in0=glo[:, :ntl * BLO, :],
                        in1=wlo[:, (l * NT + t0) * BLO:(l * NT + t0 + ntl) * BLO]
                        .to_broadcast([P, ntl * BLO, D]),
                        op=AL.mult)
                    nc.vector.tensor_tensor(
                        out=ghi[:, :ntl * BHI, :], in0=ghi[:, :ntl * BHI, :],
                        in1=whi[:, (l * NT + t0) * BHI:(l * NT + t0 + ntl) * BHI]
                        .to_broadcast([P, ntl * BHI, D]),
                        op=AL.mult)

                    for ti, t in enumerate(tiles):
                        S = sb.tile([P, BPT, P], F, tag="S")
                        tslice = tloc[:, t * BPT:(t + 1) * BPT]
                        nc.vector.tensor_tensor(
                            out=S[:],
                            in0=tslice.to_broadcast([P, BPT, P]),
                            in1=bass.AP(iota.tensor, iota[:].offset,
                                        [iota[:].ap[0], [0, BPT], [1, P]]),
                            op=AL.is_equal)
                        pa = ps_agg.tile([D, P], F, space="PSUM", tag="pa")
                        for b in range(BPT):
                            if b < BLO:
                                msg = glo[:, ti * BLO + b, :]
                            else:
                                msg = ghi[:, ti * BHI + (b - BLO), :]
                            nc.tensor.matmul(out=pa[:], lhsT=msg,
                                             rhs=S[:, b, :],
                                             start=(b == 0),
                                             stop=(b == BPT - 1))
                        # combine + MLP1
                        y0 = sb.tile([D, P], F, tag="y0")
                        nc.vector.tensor_tensor(
                            out=y0[:], in0=pa[:],
                            in1=H[:, t * P:(t + 1) * P], op=AL.add)
                        pm = ps_m.tile([D, P], F, space="PSUM", tag="pm")
                        nc.tensor.matmul(out=pm[:], lhsT=W1t[:], rhs=y0[:],
                                         start=True, stop=True)
                        n = NW if t == NT - 1 else P
                        sq = sb.tile([D, P], F, tag="sq")
                        nc.scalar.activation(
                            out=B1[:, t * P:t * P + n], in_=pm[:, :n],
                            func=AF.Copy, accum_out=s1[:, t:t + 1])
                        nc.scalar.activation(
                            out=sq[:, :n], in_=pm[:, :n], func=AF.Square,
                            accum_out=s1q[:, t:t + 1])

                # BN1 stats allreduce
                def bn_stats(sums, sq_t, idx):
                    red = sp.tile([D, 2], F, tag="red")
                    nc.vector.tensor_reduce(out=red[:, 0:1], in_=sums[:],
                                            axis=mybir.AxisListType.X,
                                            op=AL.add)
                    nc.vector.tensor_reduce(out=red[:, 1:2], in_=sq_t[:],
                                            axis=mybir.AxisListType.X,
                                            op=AL.add)
                    nc.sync.dma_start(out=st_in[idx][:], in_=red[:])
                    nc.gpsimd.collective_compute(
                        "AllReduce", AL.add, replica_groups=rg,
                        ins=[st_in[idx][:]], outs=[st_out[idx][:]])
                    st = sp.tile([D, 2], F, tag="st")
                    nc.sync.dma_start(out=st[:], in_=st_out[idx][:])
                    mean = sp.tile([D, 1], F, tag="mean")
                    ex2 = sp.tile([D, 1], F, tag="ex2")
                    nc.scalar.activation(out=mean[:], in_=st[:, 0:1],
                                         func=AF.Copy, scale=1.0 / NODES)
                    nc.scalar.activation(out=ex2[:], in_=st[:, 1:2],
                                         func=AF.Copy, scale=1.0 / NODES)
                    var = sp.tile([D, 1], F, tag="var")
                    nc.vector.tensor_tensor(out=var[:], in0=mean[:],
                                            in1=mean[:], op=AL.mult)
                    nc.vector.tensor_tensor(out=var[:], in0=ex2[:],
                                            in1=var[:], op=AL.subtract)
                    nc.vector.tensor_tensor(out=var[:], in0=var[:],
                                            in1=epsc[:], op=AL.add)
                    std = sp.tile([D, 1], F, tag="std")
                    nc.scalar.activation(out=std[:], in_=var[:], func=AF.Sqrt,
                                         bias=0.0)
                    rstd = sp.tile([D, 1], F, tag="rstd")
                    nc.vector.reciprocal(rstd[:], std[:])
                    gcol = 4 * l + (0 if idx % 2 == 0 else 2)
                    scl = sp.tile([D, 1], F, tag="scl")
                    nc.vector.tensor_tensor(out=scl[:], in0=gb[:, gcol:gcol + 1],
                                            in1=rstd[:], op=AL.mult)
                    tmp = sp.tile([D, 1], F, tag="tmp")
                    nc.vector.tensor_tensor(out=tmp[:], in0=mean[:],
                                            in1=scl[:], op=AL.mult)
                    bia = sp.tile([D, 1], F, tag="bia")
                    nc.vector.tensor_tensor(out=bia[:],
                                            in0=gb[:, gcol + 1:gcol + 2],
                                            in1=tmp[:], op=AL.subtract)
                    return scl, bia

                sc1, bi1 = bn_stats(s1, s1q, 2 * l)

                # y1n = relu(BN1(y1)); y2 = W2.T @ y1n, stats
                s2 = sp.tile([D, 16], F, tag="s2")
                s2q = sp.tile([D, 16], F, tag="s2q")
                nch = (NLOC + CHUNK - 1) // CHUNK
                for ci in range(nch):
                    c0 = ci * CHUNK
                    c1 = min(c0 + CHUNK, NLOC)
                    ca = min(c1, NSH)                # apply-BN limit
                    if ca > c0:
                        nc.scalar.activation(
                            out=B2[:, c0:ca], in_=B1[:, c0:ca], func=AF.Relu,
                            bias=bi1[:], scale=sc1[:])
                    pm2 = ps_m2.tile([D, CHUNK], F, space="PSUM", tag="pm2")
                    nc.tensor.matmul(out=pm2[:, :c1 - c0], lhsT=W2t[:],
                                     rhs=B2[:, c0:c1], start=True, stop=True)
                    sq2 = sb.tile([D, CHUNK], F, tag="sq2")
                    nc.scalar.activation(
                        out=B1[:, c0:c1], in_=pm2[:, :c1 - c0], func=AF.Copy,
                        accum_out=s2[:, ci:ci + 1])
                    nc.scalar.activation(
                        out=sq2[:, :c1 - c0], in_=pm2[:, :c1 - c0],
                        func=AF.Square, accum_out=s2q[:, ci:ci + 1])

                sc2, bi2 = bn_stats(s2[:, :nch], s2q[:, :nch], 2 * l + 1)

                # h_next = relu(BN2(y2)), transpose, store / allgather
                for t in range(NT):
                    n = NW if t == NT - 1 else P
                    nc.scalar.activation(
                        out=H[:, t * P:t * P + n], in_=B1[:, t * P:t * P + n],
                        func=AF.Relu, bias=bi2[:], scale=sc2[:])
                    ptr = ps_tr.tile([P, D], F, space="PSUM", tag="ptr")
                    nc.tensor.transpose(out=ptr[:],
                                        in_=H[:, t * P:(t + 1) * P],
                                        identity=ident[:])
                    stg = sb.tile([P, D], F, tag="stg")
                    nc.scalar.activation(out=stg[:], in_=ptr[:], func=AF.Copy)
                    dst = h_out if l == NL - 1 else cc_in[l]
                    nc.sync.dma_start(out=dst[t * P:(t + 1) * P, :],
                                      in_=stg[:])
                if l < NL - 1:
                    nc.gpsimd.collective_compute(
                        "AllGather", AL.bypass, replica_groups=rg,
                        ins=[cc_in[l][:]], outs=[cc_out[l][:]])

    nc.compile()
    return nc


def _get_nc(BLO, BHI):
    if (BLO, BHI) not in _cache:
        _cache[(BLO, BHI)] = _build(BLO, BHI)
    return _cache[(BLO, BHI)]


def kernel(x, edge_index, edge_weight, mask_teams, eps, W1, b1, g1, beta1,
           W2, b2, g2, beta2, _trace=False):
    from concourse.bass_utils import run_bass_kernel_spmd

    x = np.asarray(x, np.float32)
    eps = np.asarray(eps, np.float32)
    plan = _plan(np.asarray(edge_index), np.asarray(edge_weight), eps)
    BLO, BHI = plan["BLO"], plan["BHI"]
    table, _ = _tableize(x)
    Ws, gb = _weights(eps, np.asarray(W1), np.asarray(W2), np.asarray(g1),
                      np.asarray(beta1), np.asarray(g2), np.asarray(beta2))
    iota = np.broadcast_to(np.arange(P, dtype=np.float32), (P, P)).copy()

    in_maps = []
    for c in range(NCORES):
        pc = plan["cores"][c]
        in_maps.append({
            "x_table": table, "xT_own": table[c * NLOC:(c + 1) * NLOC].T.copy(),
            "idxlo": pc["idxlo"], "idxhi": pc["idxhi"],
            "wlo": pc["wlo"], "whi": pc["whi"], "tloc": pc["tgtloc"],
            "iota": iota, "Ws": Ws, "gb": gb,
        })

    nc = _get_nc(BLO, BHI)
    res = run_bass_kernel_spmd(nc, in_maps, list(range(NCORES)), trace=_trace)
    full = np.concatenate([res.results[c]["h_out"][:NSH]
                           for c in range(NCORES)], 0)
    out = full[np.asarray(mask_teams)]
    if _trace:
        kernel._last = res
    return out


# revision 7
# speedup vs baseline: 13.6444x; 13.6444x over previous
"""GIN message-passing (CentralityChannel) on 8 trn2 NeuronCores.

Strategy (graph/data parallel per sharding hint):
  - Nodes sharded: core c owns rows [c*6250, (c+1)*6250), padded to 6272 = 49
    tiles of 128. The full node-feature table h [50176, 64] is replicated in
    every core's DRAM and rebuilt each layer with an AllGather.
  - Edges sharded by TARGET core. Per target tile (128 nodes), edges are
    grouped into blocks of 128, split by source-table half (dma_gather uses
    int16 indices < 32768), zero-weight padded to static block counts
    (BLO lo-blocks + BHI hi-blocks per tile, same for every core).
  - Per block: gathered rows [128e, 64f] are weighted (DVE), a one-hot
    S[e, n] = (tgtloc[e] == n) is built on DVE, and PE accumulates
    aggT[64f, 128n] += msg.T @ S in PSUM == segment-sum of w*h[src].
  - GIN combine is folded: edge weights are pre-divided by (1+eps_l) on the
    host and W1_l is pre-multiplied by (1+eps_l), so y0 = hT_own + aggT.
  - MLP runs feature-major: y1T = W1'.T @ y0. BatchNorm1d (training mode)
    needs global batch stats: per-channel sum/sumsq accumulate via ACT
    accum_out, AllReduce [64, 2] across cores, then BN+bias+ReLU is one ACT
    op with per-partition scale/bias. MLP biases cancel inside BatchNorm and
    are dropped.
  - New h shard is transposed back node-major via PE and AllGathered into the
    next layer's table. The final layer skips the AllGather; the host
    assembles shards and applies mask_teams.
"""

import sys

sys.path.insert(0, "/opt/trn_rl_repo")
import numpy as np

NODES, D, NL = 50000, 64, 3
NCORES = 8
NSH = NODES // NCORES            # 6250
P = 128
NT = (NSH + P - 1) // P          # 49
NLOC = NT * P                    # 6272
TB = NCORES * NLOC               # 50176
HALF = TB // 2                   # 25088
BN_EPS = 1e-5
G = 4                            # target tiles per dma_gather call
CHUNK = 512                      # node cols per MLP2 matmul

_CALLS = [list(range(s, min(s + G, NT))) for s in range(0, NT, G)]


def _plan(edge_index, edge_weight, eps):
    """Host preprocessing: shard/sort/pad edges into the static block layout."""
    src = edge_index[0].astype(np.int64)
    tgt = edge_index[1].astype(np.int64)
    w = edge_weight.astype(np.float32)
    assert np.all(np.abs(1.0 + eps) > 1e-6), "eps == -1 unsupported"

    src_row = (src // NSH) * NLOC + (src % NSH)     # row in padded table
    c_tgt = tgt // NSH
    r = tgt % NSH
    tile = r // P
    lane = (r % P).astype(np.float32)               # one-hot lane value
    half = (src_row >= HALF).astype(np.int64)

    key = (c_tgt * NT + tile) * 2 + half
    order = np.argsort(key, kind="stable")
    counts = np.bincount(key, minlength=NCORES * NT * 2)
    starts = np.zeros_like(counts)
    starts[1:] = np.cumsum(counts)[:-1]
    q = np.arange(len(src)) - starts[key[order]]    # rank within group

    BLO = int(np.ceil(counts.reshape(-1, 2)[:, 0].max() / P))
    BHI = int(np.ceil(counts.reshape(-1, 2)[:, 1].max() / P))
    BPT = BLO + BHI

    so, ho, co, to = src_row[order], half[order], c_tgt[order], tile[order]
    lo_, wo = lane[order], w[order]
    b = q // P
    p = q % P
    bcount = np.where(ho == 0, BLO, BHI)
    assert np.all(b < bcount), "block overflow; BLO/BHI too small"

    cores = []
    for c in range(NCORES):
        m = co == c
        tokv = {0: np.zeros(NT * BLO * P, np.int64),
                1: np.zeros(NT * BHI * P, np.int64)}
        wtok = {0: np.zeros(NT * BLO * P, np.float32),
                1: np.zeros(NT * BHI * P, np.float32)}
        ttok = np.zeros(NT * BPT * P, np.float32)
        for h, bp in ((0, BLO), (1, BHI)):
            mh = m & (ho == h)
            pos = (to[mh] * bp + b[mh]) * P + p[mh]
            tokv[h][pos] = so[mh] - h * HALF
            wtok[h][pos] = wo[mh]
            bb = b[mh] + (BLO if h else 0)
            ttok[(to[mh] * BPT + bb) * P + p[mh]] = lo_[mh]

        def wrap(tv, bp):
            outs = []
            for tiles in _CALLS:
                t0, t1 = tiles[0], tiles[-1] + 1
                seg = tv[t0 * bp * P:t1 * bp * P]
                outs.append(seg.reshape(-1, 16).T)
            wr = np.concatenate(outs, axis=1).astype(np.int16)
            return np.tile(wr, (8, 1))              # replicate to 128 rows

        wlo = np.stack([wtok[0] / (1.0 + eps[l]) for l in range(NL)])
        whi = np.stack([wtok[1] / (1.0 + eps[l]) for l in range(NL)])
        cores.append(dict(
            idxlo=wrap(tokv[0], BLO), idxhi=wrap(tokv[1], BHI),
            wlo=np.concatenate([a.reshape(NT * BLO, P).T for a in wlo], 1),
            whi=np.concatenate([a.reshape(NT * BHI, P).T for a in whi], 1),
            tgtloc=ttok.reshape(NT * BPT, P).T.copy(),
            tokv=tokv, wtok=wtok, ttokf=ttok,
        ))
    return dict(BLO=BLO, BHI=BHI, BPT=BPT, cores=cores)


def _tableize(x):
    rows = (np.arange(NODES) // NSH) * NLOC + np.arange(NODES) % NSH
    tb = np.zeros((TB, D), np.float32)
    tb[rows] = x
    return tb, rows


def _weights(eps, W1, W2, g1, beta1, g2, beta2):
    ws = []
    for l in range(NL):
        ws.append(((1.0 + eps[l]) * W1[l]).astype(np.float32))
        ws.append(W2[l].astype(np.float32))
    Ws = np.concatenate(ws, 0)                       # [NL*2*64, 64]
    gb = np.stack(sum([[g1[l], beta1[l], g2[l], beta2[l]] for l in range(NL)],
                      []), 1).astype(np.float32)     # [64, 12]
    return Ws, gb


def mirror(x, edge_index, edge_weight, mask_teams, eps, W1, b1, g1, beta1,
           W2, b2, g2, beta2):
    """Numpy mirror of the exact device computation (for validation)."""
    plan = _plan(np.asarray(edge_index), np.asarray(edge_weight),
                 np.asarray(eps))
    BLO, BHI, BPT = plan["BLO"], plan["BHI"], plan["BPT"]
    table, rows = _tableize(np.asarray(x))
    Ws, gb = _weights(eps, W1, W2, g1, beta1, g2, beta2)
    H = [table[c * NLOC:(c + 1) * NLOC].T.copy() for c in range(NCORES)]

    for l in range(NL):
        aggs = []
        for c in range(NCORES):
            pc = plan["cores"][c]
            msil = table[:HALF][pc["tokv"][0]] * \
                (pc["wtok"][0] / (1 + eps[l]))[:, None]
            msih = table[HALF:][pc["tokv"][1]] * \
                (pc["wtok"][1] / (1 + eps[l]))[:, None]
            mlo = msil.reshape(NT, BLO, P, D)
            mhi = msih.reshape(NT, BHI, P, D)
            msg = np.concatenate([mlo, mhi], 1)       # [NT, BPT, P, D]
            tl = pc["ttokf"].reshape(NT, BPT, P)
            S = (tl[..., None] == np.arange(P, dtype=np.float32)).astype(
                np.float32)                           # [NT, BPT, P, Pn]
            agg = np.einsum("tbpd,tbpn->dtn", msg, S).reshape(D, NLOC)
            aggs.append(agg)
        # MLP, feature-major, with cross-core BN
        y1s = []
        for c in range(NCORES):
            y0 = H[c] + aggs[c]
            y1s.append(Ws[2 * l * D:(2 * l + 1) * D].T @ y0)
        s1 = sum(y[:, :NSH].sum(1) for y in y1s)
        s1q = sum((y[:, :NSH] ** 2).sum(1) for y in y1s)
        mu, ex2 = s1 / NODES, s1q / NODES
        sc1 = gb[:, 4 * l + 0] / np.sqrt(ex2 - mu ** 2 + BN_EPS)
        bi1 = gb[:, 4 * l + 1] - mu * sc1
        y2s = []
        for c in range(NCORES):
            y1n = np.zeros_like(y1s[c])
            y1n[:, :NSH] = np.maximum(
                y1s[c][:, :NSH] * sc1[:, None] + bi1[:, None], 0)
            y2s.append(Ws[(2 * l + 1) * D:(2 * l + 2) * D].T @ y1n)
        s2 = sum(y[:, :NSH].sum(1) for y in y2s)
        s2q = sum((y[:, :NSH] ** 2).sum(1) for y in y2s)
        mu2, ex22 = s2 / NODES, s2q / NODES
        sc2 = gb[:, 4 * l + 2] / np.sqrt(ex22 - mu2 ** 2 + BN_EPS)
        bi2 = gb[:, 4 * l + 3] - mu2 * sc2
        for c in range(NCORES):
            hn = np.zeros_like(y2s[c])
            hn[:, :NSH] = np.maximum(
                y2s[c][:, :NSH] * sc2[:, None] + bi2[:, None], 0)
            H[c] = hn
            table[c * NLOC:(c + 1) * NLOC] = hn.T
    full = np.concatenate([H[c].T[:NSH] for c in range(NCORES)], 0)
    return full[np.asarray(mask_teams)]


# ---------------------------------------------------------------------------
# Device program
# ---------------------------------------------------------------------------
_cache = {}


def _build(BLO, BHI, stage=5):
    from concourse import bass, bacc, mybir, tile
    from concourse.masks import make_identity

    F = mybir.dt.float32
    I16 = mybir.dt.int16
    BPT = BLO + BHI
    AL = mybir.AluOpType
    AF = mybir.ActivationFunctionType

    nc = bacc.Bacc(num_devices=NCORES, num_swdge_queues=2)
    x_table = nc.declare_dram_parameter("x_table", [TB, D], F, isOutput=False)
    xT_own = nc.declare_dram_parameter("xT_own", [D, NLOC], F, isOutput=False)
    idxlo = nc.declare_dram_parameter("idxlo", [P, NT * BLO * 8], I16, False)
    idxhi = nc.declare_dram_parameter("idxhi", [P, NT * BHI * 8], I16, False)
    wlo_in = nc.declare_dram_parameter("wlo", [P, NL * NT * BLO], F, False)
    whi_in = nc.declare_dram_parameter("whi", [P, NL * NT * BHI], F, False)
    tloc_in = nc.declare_dram_parameter("tloc", [P, NT * BPT], F, False)
    iota_in = nc.declare_dram_parameter("iota", [P, P], F, False)
    Ws_in = nc.declare_dram_parameter("Ws", [NL * 2 * D, D], F, False)
    gb_in = nc.declare_dram_parameter("gb", [D, 4 * NL], F, False)
    h_out = nc.declare_dram_parameter("h_out", [NLOC, D], F, isOutput=True)

    cc_in = [nc.dram_tensor(f"cc_in{l}", [NLOC, D], F) for l in range(NL - 1)]
    cc_out = [nc.dram_tensor(f"cc_out{l}", [TB, D], F, addr_space="Shared")
              for l in range(NL - 1)]
    st_in = [nc.dram_tensor(f"st_in{i}", [D, 2], F) for i in range(2 * NL)]
    st_out = [nc.dram_tensor(f"st_out{i}", [D, 2], F, addr_space="Shared")
              for i in range(2 * NL)]
    rg = [list(range(NCORES))]

    with tile.TileContext(nc) as tc:
        with (
            tc.tile_pool(name="persist", bufs=1) as pp,
            tc.tile_pool(name="gat", bufs=2) as gp,
            tc.tile_pool(name="sb", bufs=3) as sb,
            tc.tile_pool(name="small", bufs=4) as sp,
            tc.tile_pool(name="ps_agg", bufs=2, space="PSUM") as ps_agg,
            tc.tile_pool(name="ps_m", bufs=2, space="PSUM") as ps_m,
            tc.tile_pool(name="ps_m2", bufs=2, space="PSUM") as ps_m2,
            tc.tile_pool(name="ps_tr", bufs=2, space="PSUM") as ps_tr,
        ):
            # resident tiles
            H = pp.tile([D, NLOC], F)
            B1 = pp.tile([D, NLOC], F)
            B2 = pp.tile([D, NLOC], F)
            ilo = pp.tile([P, NT * BLO * 8], I16)
            ihi = pp.tile([P, NT * BHI * 8], I16)
            wlo = pp.tile([P, NL * NT * BLO], F)
            whi = pp.tile([P, NL * NT * BHI], F)
            tloc = pp.tile([P, NT * BPT], F)
            iota = pp.tile([P, P], F)
            gb = pp.tile([D, 4 * NL], F)
            ident = pp.tile([D, D], F)

            nc.sync.dma_start(out=H[:], in_=xT_own[:])
            nc.sync.dma_start(out=ilo[:], in_=idxlo[:])
            nc.sync.dma_start(out=ihi[:], in_=idxhi[:])
            nc.sync.dma_start(out=wlo[:], in_=wlo_in[:])
            nc.sync.dma_start(out=whi[:], in_=whi_in[:])
            nc.sync.dma_start(out=tloc[:], in_=tloc_in[:])
            nc.sync.dma_start(out=iota[:], in_=iota_in[:])
            nc.sync.dma_start(out=gb[:], in_=gb_in[:])
            make_identity(nc, ident[:])
            nc.vector.memset(B1[:], 0.0)
            nc.vector.memset(B2[:], 0.0)
            epsc = pp.tile([D, 1], F)
            nc.vector.memset(epsc[:], BN_EPS)

            NW = NSH - (NT - 1) * P                  # 106 real cols, last tile

            for l in range(NL):
                tab = x_table if l == 0 else cc_out[l - 1]
                W1t = sp.tile([D, D], F, tag="w1")
                W2t = sp.tile([D, D], F, tag="w2")
                nc.sync.dma_start(out=W1t[:], in_=Ws_in[2 * l * D:(2 * l + 1) * D, :])
                nc.sync.dma_start(out=W2t[:], in_=Ws_in[(2 * l + 1) * D:(2 * l + 2) * D, :])
                s1 = sp.tile([D, NT], F, tag="s1")
                s1q = sp.tile([D, NT], F, tag="s1q")

                for ci, tiles in enumerate(_CALLS):
                    ntl = len(tiles)
                    t0 = tiles[0]
                    glo = gp.tile([P, G * BLO, D], F, tag="glo")
                    ghi = gp.tile([P, G * BHI, D], F, tag="ghi")
                    if stage < 1:
                        continue
                    nc.gpsimd.dma_gather(
                        out_ap=glo[:, :ntl * BLO, :], in_ap=tab[0:HALF, :],
                        idxs_ap=ilo[:, t0 * BLO * 8:(t0 + ntl) * BLO * 8],
                        num_idxs=ntl * BLO * P, num_idxs_reg=ntl * BLO * P,
                        elem_size=D, single_packet=False)
                    nc.gpsimd.dma_gather(
                        out_ap=ghi[:, :ntl * BHI, :], in_ap=tab[HALF:TB, :],
                        idxs_ap=ihi[:, t0 * BHI * 8:(t0 + ntl) * BHI * 8],
                        num_idxs=ntl * BHI * P, num_idxs_reg=ntl * BHI * P,
                        elem_size=D, single_packet=False, queue_num=1)
                    # weight the messages (in place)
                    if stage < 2:
                        continue
                    nc.vector.tensor_tensor(
                        out=glo[:, :ntl * BLO, :], in0=glo[:, :ntl * BLO, :],
                        in1=wlo[:, (l * NT + t0) * BLO:(l * NT + t0 + ntl) * BLO]
                        .to_broadcast([P, ntl * BLO, D]),
                        op=AL.mult)
                    nc.vector.tensor_tensor(
                        out=ghi[:, :ntl * BHI, :], in0=ghi[:, :ntl * BHI, :],
                        in1=whi[:, (l * NT + t0) * BHI:(l * NT + t0 + ntl) * BHI]
                        .to_broadcast([P, ntl * BHI, D]),
                        op=AL.mult)

                    if stage < 3:
                        continue
                    for ti, t in enumerate(tiles):
                        S = sb.tile([P, BPT, P], F, tag="S")
                        tslice = tloc[:, t * BPT:(t + 1) * BPT]
                        nc.vector.tensor_tensor(
                            out=S[:],
                            in0=tslice.to_broadcast([P, BPT, P]),
                            in1=bass.AP(iota.tensor, iota[:].offset,
                                        [iota[:].ap[0], [0, BPT], [1, P]]),
                            op=AL.is_equal)
                        if stage < 4:
                            continue
                        pa = ps_agg.tile([D, P], F, space="PSUM", tag="pa")
                        for b in range(BPT):
                            if b < BLO:
                                msg = glo[:, ti * BLO + b, :]
                            else:
                                msg = ghi[:, ti * BHI + (b - BLO), :]
                            nc.tensor.matmul(out=pa[:], lhsT=msg,
                                             rhs=S[:, b, :],
                                             start=(b == 0),
                                             stop=(b == BPT - 1))
                        # combine + MLP1
                        y0 = sb.tile([D, P], F, tag="y0")
                        nc.vector.tensor_tensor(
                            out=y0[:], in0=pa[:],
                            in1=H[:, t * P:(t + 1) * P], op=AL.add)
                        pm = ps_m.tile([D, P], F, space="PSUM", tag="pm")
                        nc.tensor.matmul(out=pm[:], lhsT=W1t[:], rhs=y0[:],
                                         start=True, stop=True)
                        n = NW if t == NT - 1 else P
                        sq = sb.tile([D, P], F, tag="sq")
                        nc.scalar.activation(
                            out=B1[:, t * P:t * P + n], in_=pm[:, :n],
                            func=AF.Copy, accum_out=s1[:, t:t + 1])
                        nc.scalar.activation(
                            out=sq[:, :n], in_=pm[:, :n], func=AF.Square,
                            accum_out=s1q[:, t:t + 1])

                if stage < 5:
                    continue
                # BN1 stats allreduce
                def bn_stats(sums, sq_t, idx):
                    red = sp.tile([D, 2], F, tag="red")
                    nc.vector.tensor_reduce(out=red[:, 0:1], in_=sums[:],
                                            axis=mybir.AxisListType.X,
                                            op=AL.add)
                    nc.vector.tensor_reduce(out=red[:, 1:2], in_=sq_t[:],
                                            axis=mybir.AxisListType.X,
                                            op=AL.add)
                    nc.sync.dma_start(out=st_in[idx][:], in_=red[:])
                    nc.gpsimd.collective_compute(
                        "AllReduce", AL.add, replica_groups=rg,
                        ins=[st_in[idx][:]], outs=[st_out[idx][:]])
                    st = sp.tile([D, 2], F, tag="st")
                    nc.sync.dma_start(out=st[:], in_=st_out[idx][:])
                    mean = sp.tile([D, 1], F, tag="mean")
                    ex2 = sp.tile([D, 1], F, tag="ex2")
                    nc.scalar.activation(out=mean[:], in_=st[:, 0:1],
                                         func=AF.Copy, scale=1.0 / NODES)
                    nc.scalar.activation(out=ex2[:], in_=st[:, 1:2],
                                         func=AF.Copy, scale=1.0 / NODES)
                    var = sp.tile([D, 1], F, tag="var")
                    nc.vector.tensor_tensor(out=var[:], in0=mean[:],
                                            in1=mean[:], op=AL.mult)
                    nc.vector.tensor_tensor(out=var[:], in0=ex2[:],
                                            in1=var[:], op=AL.subtract)
                    nc.vector.tensor_tensor(out=var[:], in0=var[:],
                                            in1=epsc[:], op=AL.add)
                    std = sp.tile([D, 1], F, tag="std")
                    nc.scalar.activation(out=std[:], in_=var[:], func=AF.Sqrt,
                                         bias=0.0)
                    rstd = sp.tile([D, 1], F, tag="rstd")
                    nc.vector.reciprocal(rstd[:], std[:])
                    gcol = 4 * l + (0 if idx % 2 == 0 else 2)
                    scl = sp.tile([D, 1], F, tag="scl")
                    nc.vector.tensor_tensor(out=scl[:], in0=gb[:, gcol:gcol + 1],
                                            in1=rstd[:], op=AL.mult)
                    tmp = sp.tile([D, 1], F, tag="tmp")
                    nc.vector.tensor_tensor(out=tmp[:], in0=mean[:],
                                            in1=scl[:], op=AL.mult)
                    bia = sp.tile([D, 1], F, tag="bia")
                    nc.vector.tensor_tensor(out=bia[:],
                                            in0=gb[:, gcol + 1:gcol + 2],
                                            in1=tmp[:], op=AL.subtract)
                    return scl, bia

                sc1, bi1 = bn_stats(s1, s1q, 2 * l)

                # y1n = relu(BN1(y1)); y2 = W2.T @ y1n, stats
                s2 = sp.tile([D, 16], F, tag="s2")
                s2q = sp.tile([D, 16], F, tag="s2q")
                nch = (NLOC + CHUNK - 1) // CHUNK
                for ci in range(nch):
                    c0 = ci * CHUNK
                    c1 = min(c0 + CHUNK, NLOC)
                    ca = min(c1, NSH)                # apply-BN limit
                    if ca > c0:
                        nc.scalar.activation(
                            out=B2[:, c0:ca], in_=B1[:, c0:ca], func=AF.Relu,
                            bias=bi1[:], scale=sc1[:])
                    pm2 = ps_m2.tile([D, CHUNK], F, space="PSUM", tag="pm2")
                    nc.tensor.matmul(out=pm2[:, :c1 - c0], lhsT=W2t[:],
                                     rhs=B2[:, c0:c1], start=True, stop=True)
                    sq2 = sb.tile([D, CHUNK], F, tag="sq2")
                    nc.scalar.activation(
                        out=B1[:, c0:c1], in_=pm2[:, :c1 - c0], func=AF.Copy,
                        accum_out=s2[:, ci:ci + 1])
                    nc.scalar.activation(
                        out=sq2[:, :c1 - c0], in_=pm2[:, :c1 - c0],
                        func=AF.Square, accum_out=s2q[:, ci:ci + 1])

                sc2, bi2 = bn_stats(s2[:, :nch], s2q[:, :nch], 2 * l + 1)

                # h_next = relu(BN2(y2)), transpose, store / allgather
                for t in range(NT):
                    n = NW if t == NT - 1 else P
                    nc.scalar.activation(
                        out=H[:, t * P:t * P + n], in_=B1[:, t * P:t * P + n],
                        func=AF.Relu, bias=bi2[:], scale=sc2[:])
                    ptr = ps_tr.tile([P, D], F, space="PSUM", tag="ptr")
                    nc.tensor.transpose(out=ptr[:],
                                        in_=H[:, t * P:(t + 1) * P],
                                        identity=ident[:])
                    stg = sb.tile([P, D], F, tag="stg")
                    nc.scalar.activation(out=stg[:], in_=ptr[:], func=AF.Copy)
                    dst = h_out if l == NL - 1 else cc_in[l]
                    nc.sync.dma_start(out=dst[t * P:(t + 1) * P, :],
                                      in_=stg[:])
                if l < NL - 1:
                    nc.gpsimd.collective_compute(
                        "AllGather", AL.bypass, replica_groups=rg,
                        ins=[cc_in[l][:]], outs=[cc_out[l][:]])
            if stage < 5:
                nc.sync.dma_start(out=h_out[0:D, :], in_=H[:, 0:D])

    nc.compile()
    return nc


def _get_nc(BLO, BHI):
    if (BLO, BHI) not in _cache:
        _cache[(BLO, BHI)] = _build(BLO, BHI)
    return _cache[(BLO, BHI)]


def kernel(x, edge_index, edge_weight, mask_teams, eps, W1, b1, g1, beta1,
           W2, b2, g2, beta2, _trace=False):
    from concourse.bass_utils import run_bass_kernel_spmd

    x = np.asarray(x, np.float32)
    eps = np.asarray(eps, np.float32)
    plan = _plan(np.asarray(edge_index), np.asarray(edge_weight), eps)
    BLO, BHI = plan["BLO"], plan["BHI"]
    table, _ = _tableize(x)
    Ws, gb = _weights(eps, np.asarray(W1), np.asarray(W2), np.asarray(g1),
                      np.asarray(beta1), np.asarray(g2), np.asarray(beta2))
    iota = np.broadcast_to(np.arange(P, dtype=np.float32), (P, P)).copy()

    in_maps = []
    for c in range(NCORES):
        pc = plan["cores"][c]
        in_maps.append({
            "x_table": table, "xT_own": table[c * NLOC:(c + 1) * NLOC].T.copy(),
            "idxlo": pc["idxlo"], "idxhi": pc["idxhi"],
            "wlo": pc["wlo"], "whi": pc["whi"], "tloc": pc["tgtloc"],
            "iota": iota, "Ws": Ws, "gb": gb,
        })

    nc = _get_nc(BLO, BHI)
    res = run_bass_kernel_spmd(nc, in_maps, list(range(NCORES)), trace=_trace)
    full = np.concatenate([res.results[c]["h_out"][:NSH]
                           for c in range(NCORES)], 0)
    out = full[np.asarray(mask_teams)]
    if _trace:
        kernel._last = res
    return out
